# revision 8
# baseline (speedup 1.0000x reference)
"""Bass/Trainium2 kernel for nn_Attn (dot+affect attention over encoder outputs).

Computation (per batch b):
  e[b, l] = h[b] . enc[l, b]  +  (h[b] @ affect) . emb[l, b]
  out[b, 0, :] = softmax(e[b, :])

Strategy: data-parallel over batch (8 batches per core on 8 cores). The host
packs enc and emb into one [L, BLOC*(H+A)] tensor per core so a single fused
VectorE tensor_tensor_reduce per (b, l-tile) computes the full dot product in
one pass over the streamed data (memory-bound regime). Softmax runs on a
transposed [128, 128] score tile using mask matmuls for partition-group
reductions/broadcasts.
"""

import numpy as np

import concourse.bass as bass
import concourse.tile as tile
from concourse import bacc, mybir
from concourse.bass_utils import run_bass_kernel_spmd

F32 = mybir.dt.float32
L, B, H, A = 2048, 64, 1024, 3
NCORES = 8
BLOC = B // NCORES          # batches per core
HE = H + A                  # extended hidden width (dot + affect features)
P = 128                     # SBUF partitions / l-tile height


def build_nc(l_total: int = L):
    no = l_total // P       # number of l-tiles
    cols = BLOC * no        # score columns: c = b*no + o

    nc = bacc.Bacc("TRN2", target_bir_lowering=False, debug=False)

    enc_d = nc.dram_tensor("enc", [l_total, BLOC * HE], F32, kind="ExternalInput")
    hid_d = nc.dram_tensor("hid", [BLOC, H], F32, kind="ExternalInput")
    aff_d = nc.dram_tensor("aff", [1, H * A], F32, kind="ExternalInput")
    ident_d = nc.dram_tensor("ident", [P, P], F32, kind="ExternalInput")
    ones_d = nc.dram_tensor("ones_", [1, P], F32, kind="ExternalInput")
    bm_d = nc.dram_tensor("bm", [cols, BLOC], F32, kind="ExternalInput")
    bmT_d = nc.dram_tensor("bmT", [BLOC, cols], F32, kind="ExternalInput")
    nbmT_d = nc.dram_tensor("nbmT", [BLOC, cols], F32, kind="ExternalInput")
    sel_d = nc.dram_tensor("sel", [BLOC, BLOC * P], F32, kind="ExternalInput")
    out_d = nc.dram_tensor("out", [BLOC, l_total], F32, kind="ExternalOutput")

    mult = mybir.AluOpType.mult
    add = mybir.AluOpType.add
    amax = mybir.AluOpType.max
    AX = mybir.AxisListType.X

    with tile.TileContext(nc) as tc:
        with (
            tc.tile_pool(name="const", bufs=1) as cpool,
            tc.tile_pool(name="slab", bufs=3) as spool,
            tc.tile_pool(name="scratch", bufs=2) as tpool,
            tc.tile_pool(name="ps_bc", bufs=2, space="PSUM") as ppool,
            tc.tile_pool(name="ps_sm", bufs=4, space="PSUM") as qpool,
        ):
            # ---- constants / small inputs ----
            ident = cpool.tile([P, P], F32)
            nc.sync.dma_start(ident[:], ident_d[:])
            ones = cpool.tile([1, P], F32)
            nc.sync.dma_start(ones[:], ones_d[:])
            bm = cpool.tile([cols, BLOC], F32)
            nc.sync.dma_start(bm[:], bm_d[:])
            bmT = cpool.tile([BLOC, cols], F32)
            nc.sync.dma_start(bmT[:], bmT_d[:])
            nbmT = cpool.tile([BLOC, cols], F32)
            nc.sync.dma_start(nbmT[:], nbmT_d[:])
            h_sb = cpool.tile([BLOC, H], F32)
            nc.sync.dma_start(h_sb[:], hid_d[:])
            sel = cpool.tile([BLOC, BLOC * P], F32)
            nc.sync.dma_start(sel[:], sel_d[:])
            af_sb = cpool.tile([1, H * A], F32)
            nc.sync.dma_start(af_sb[:], aff_d[:])

            # ---- ha = h @ affect  ([BLOC, A]) ----
            # broadcast affect column k across BLOC partitions, then fused
            # multiply+reduce against h.
            af_r = af_sb[:].rearrange("p (h k) -> p k h", k=A)   # [1, A, H]
            af_bc = cpool.tile([BLOC, A * H], F32)
            for k in range(A):
                for j in range(H // 512):
                    chunk = ppool.tile([BLOC, 512], F32, tag="bc", name="chunk")
                    nc.tensor.matmul(
                        chunk[:],
                        ones[0:1, 0:BLOC],
                        af_r[:, k, bass.ts(j, 512)],
                        start=True, stop=True,
                    )
                    nc.scalar.copy(af_bc[:, k * H + j * 512:k * H + (j + 1) * 512], chunk[:])
            ha_sb = cpool.tile([BLOC, A], F32)
            for k in range(A):
                scr_h = tpool.tile([BLOC, H], F32, tag="scr_h", name="scr_h")
                nc.vector.tensor_mul(scr_h[:], h_sb[:], af_bc[:, bass.ts(k, H)])
                scr_c = tpool.tile([BLOC, H], F32, tag="scr_c", name="scr_c")
                nc.scalar.activation(
                    scr_c[:], scr_h[:], mybir.ActivationFunctionType.Copy,
                    accum_out=ha_sb[:, k:k + 1],
                )

            # ---- hbext: per-b extended hidden [h[b], ha[b]] broadcast to 128 partitions ----
            hbext = cpool.tile([P, BLOC * HE], F32)
            for b in range(BLOC):
                # sel[:, b*P:(b+1)*P] is a one-hot row-b selector: the matmul
                # both selects row b of h_sb/ha_sb and broadcasts it to 128
                # partitions from base partition 0.
                eb = sel[:, bass.ts(b, P)]
                for j in range(H // 512):
                    hb_ps = ppool.tile([P, 512], F32, tag="bc", name="hb_ps")
                    nc.tensor.matmul(
                        hb_ps[:],
                        eb,
                        h_sb[:, bass.ts(j, 512)],
                        start=True, stop=True,
                    )
                    nc.scalar.copy(hbext[:, b * HE + j * 512:b * HE + (j + 1) * 512], hb_ps[:])
                hab_ps = ppool.tile([P, A], F32, tag="bc", name="hab_ps")
                nc.tensor.matmul(
                    hab_ps[:],
                    eb,
                    ha_sb[:],
                    start=True, stop=True,
                )
                nc.scalar.copy(hbext[:, b * HE + H:b * HE + H + A], hab_ps[:])

            # ---- main loop: stream enc slabs, fused dot-product into score columns ----
            scores = cpool.tile([P, cols], F32)
            enc_r = enc_d[:].rearrange("(o p) f -> o p f", p=P)   # [no, P, BLOC*HE]
            for o in range(no):
                slab = spool.tile([P, BLOC * HE], F32, tag="slab", name="slab")
                nc.sync.dma_start(slab[:], enc_r[o])
                for b in range(BLOC):
                    # fused dot product: DVE elementwise multiply, then the
                    # ScalarE Copy-with-accumulate reduces along the free dim
                    # (TENSOR_TENSOR_REDUCE faults on this HW, so split).
                    prod = tpool.tile([P, HE], F32, tag="prod", name="prod", bufs=3)
                    nc.vector.tensor_mul(
                        prod[:],
                        slab[:, b * HE:(b + 1) * HE],
                        hbext[:, b * HE:(b + 1) * HE],
                    )
                    cpy = tpool.tile([P, HE], F32, tag="cpy", name="cpy", bufs=3)
                    nc.scalar.activation(
                        cpy[:], prod[:], mybir.ActivationFunctionType.Copy,
                        accum_out=scores[:, b * no + o:b * no + o + 1],
                    )

            # ---- softmax over l per batch, on transposed scores ----
            # scoresT[c, li] with c = b*no + o holds e[b, o*128 + li]
            scT_ps = qpool.tile([cols, P], F32, tag="sm", name="scT_ps")
            nc.tensor.transpose(scT_ps[:], scores[:], ident[:])
            scT = cpool.tile([cols, P], F32)
            nc.scalar.copy(scT[:], scT_ps[:])

            rowmax = cpool.tile([cols, 1], F32)
            nc.vector.tensor_reduce(rowmax[:], scT[:], axis=AX, op=amax)
            # transpose rowmax -> [1, cols] (identity matmul; exact for our needs)
            rmT_ps = qpool.tile([1, cols], F32, tag="sm", name="rmT_ps")
            nc.tensor.matmul(rmT_ps[:], rowmax[:], ident[0:cols, 0:cols], start=True, stop=True)
            rm_sb = cpool.tile([1, cols], F32)
            nc.scalar.copy(rm_sb[:], rmT_ps[:])
            # per-batch max over the `no` tiles
            bmax = cpool.tile([1, BLOC], F32)
            nc.vector.tensor_reduce(
                bmax[:], rm_sb[:].rearrange("p (b o) -> p b o", b=BLOC), axis=AX, op=amax
            )
            # -> column vector [BLOC, 1]
            bcol_ps = qpool.tile([BLOC, 1], F32, tag="sm", name="bcol_ps")
            nc.tensor.matmul(bcol_ps[:], bmax[:], ones[0:1, 0:1], start=True, stop=True)
            bcol = cpool.tile([BLOC, 1], F32)
            nc.scalar.copy(bcol[:], bcol_ps[:])
            # negated broadcast back to all (b, o) rows: negm[c] = -max_b
            negm_ps = qpool.tile([cols, 1], F32, tag="sm", name="negm_ps")
            nc.tensor.matmul(negm_ps[:], nbmT[:], bcol[:], start=True, stop=True)
            negm = cpool.tile([cols, 1], F32)
            nc.scalar.copy(negm[:], negm_ps[:])

            # exp(scores - max) with fused per-row sums
            expT = cpool.tile([cols, P], F32)
            rowsum = cpool.tile([cols, 1], F32)
            nc.scalar.activation(
                expT[:], scT[:], mybir.ActivationFunctionType.Exp,
                bias=negm[:], scale=1.0, accum_out=rowsum[:],
            )
            # per-batch sum over the `no` tiles: ssum[b] = sum_c(bm[c,b]*rowsum[c])
            ssum_ps = qpool.tile([BLOC, 1], F32, tag="sm", name="ssum_ps")
            nc.tensor.matmul(ssum_ps[:], bm[:], rowsum[:], start=True, stop=True)
            rsum = cpool.tile([BLOC, 1], F32)
            nc.vector.reciprocal(rsum[:], ssum_ps[:])
            # broadcast 1/sum back to rows
            rbc_ps = qpool.tile([cols, 1], F32, tag="sm", name="rbc_ps")
            nc.tensor.matmul(rbc_ps[:], bmT[:], rsum[:], start=True, stop=True)
            rbc = cpool.tile([cols, 1], F32)
            nc.scalar.copy(rbc[:], rbc_ps[:])

            outT = cpool.tile([cols, P], F32)
            nc.vector.tensor_scalar_mul(outT[:], expT[:], rbc[:, 0:1])
            nc.sync.dma_start(out_d[:].rearrange("b (o li) -> (b o) li", o=no), outT[:])

    nc.compile()
    return nc


def make_aux(l_total: int = L):
    no = l_total // P
    cols = BLOC * no
    ident = np.eye(P, dtype=np.float32)
    ones_ = np.ones((1, P), dtype=np.float32)
    bmT = np.zeros((BLOC, cols), dtype=np.float32)
    for b in range(BLOC):
        bmT[b, b * no:(b + 1) * no] = 1.0
    sel = np.zeros((BLOC, BLOC * P), dtype=np.float32)
    for b in range(BLOC):
        sel[b, b * P:(b + 1) * P] = 1.0
    return {
        "ident": ident,
        "ones_": ones_,
        "bm": np.ascontiguousarray(bmT.T),
        "bmT": bmT,
        "nbmT": -bmT,
        "sel": sel,
    }


def make_in_maps(hidden, encoder_outputs, embedding, affect_matrix, l_total: int = L):
    aux = make_aux(l_total)
    aff = np.ascontiguousarray(affect_matrix.reshape(1, H * A), dtype=np.float32)
    in_maps = []
    for i in range(NCORES):
        bs = slice(i * BLOC, (i + 1) * BLOC)
        enc_ext = np.concatenate(
            [encoder_outputs[:, bs, :], embedding[:, bs, :]], axis=2
        ).reshape(l_total, BLOC * HE)
        in_maps.append({
            "enc": np.ascontiguousarray(enc_ext, dtype=np.float32),
            "hid": np.ascontiguousarray(hidden[0, bs, :], dtype=np.float32),
            "aff": aff,
            **aux,
        })
    return in_maps


_NC_CACHE = {}


def kernel(hidden, encoder_outputs, embedding, affect_matrix):
    hidden = np.asarray(hidden, dtype=np.float32)
    encoder_outputs = np.asarray(encoder_outputs, dtype=np.float32)
    embedding = np.asarray(embedding, dtype=np.float32)
    affect_matrix = np.asarray(affect_matrix, dtype=np.float32)

    if L not in _NC_CACHE:
        _NC_CACHE[L] = build_nc(L)
    nc = _NC_CACHE[L]
    in_maps = make_in_maps(hidden, encoder_outputs, embedding, affect_matrix, L)
    res = run_bass_kernel_spmd(nc, in_maps, list(range(NCORES))).results
    out = np.concatenate(
        [res[i]["out"].reshape(BLOC, 1, L) for i in range(NCORES)], axis=0
    )
    return out
